# revision 5
# baseline (speedup 1.0000x reference)
"""Trainium2 Bass kernel for the soft-logic-gate CA problem.

Math (per sample, grid 128x128, 4 layers):
  state' = clip( sum_m sigmoid(tg[l,m]) * prod_j g(bit_j(m), tap_j), 0, 1 )
  taps: A=state[x,y], B=state[x,y+1], C=state[x+1,y], D=state[x+1,y+1] (periodic)
  g(0,t)=1-t, g(1,t)=t;  m = bA*8 + bB*4 + bC*2 + bD.

4-D multilinear interpolation of the 16 gate maps at corner (A,B,C,D).
The sigmoided gates are converted OFFLINE (host numpy, input-independent
weight preprocessing) to multilinear-polynomial coefficients via the
Moebius transform (c[m] -= c[m-bit]); the device evaluates each layer
with a Horner butterfly of fp16 tensor_tensor ops (A, then B, C, D).

Layout: partition = grid row (128).  State is parity planes (b, t, k),
t=0 even grid cols, t=1 odd, with plane width TK=65: column 64 of each
plane duplicates column 0, so the periodic shift y+1 becomes clean
unit-stride views (t=0 -> O[0..64], t=1 -> E[1..64]) with no wrap-column
split ops.  The dup columns are maintained by a tiny second clamp op per
layer; coefficients carry matching dup columns from the host.  Row
shifts (x+1): layer 0 reads host-prepped rolled copies of x; layers 1-3
use a PE permutation matmul (2 x 260-wide) + ACT copy-back from PSUM.
Coefficients arrive fp16 in final (m, t, k) layouts: no on-chip sigmoid,
Moebius, or casts.  Layer 0 ships only its 8 live coefficients (B=D=0
initially), layer 3 only even-column ones (only even cols are read out).

Sharding: batch 32 -> 8 cores x 4 samples (coefficients replicated).
Engines: DVE does Horner + clamps; Pool (gpsimd) takes the A-level of
sample 3 in layers 1-3; ACT does PSUM copy-backs; PE the row-shift
matmuls; DMA on the two HW-DGE queues (sync, scalar).
"""

import numpy as np

import concourse.bacc as bacc
import concourse.mybir as mybir
from concourse.ap import AP
from concourse.tile import TileContext
from concourse.bass_utils import run_bass_kernel_spmd

F32 = mybir.dt.float32
DT = mybir.dt.float16  # compute dtype
AL = mybir.AluOpType
P = 128          # partitions = grid rows
B = 4            # samples per core
Y = 128          # grid cols
K = 64           # x cols (even grid cols)
L = 4
M = 16
N_CORES = 8

TK = K + 1       # plane width incl dup col (plane[64] = plane[0])
SS = 2 * TK      # state elems per sample (E|O planes)
CH = 768         # chunk width: [g0 half (256) | x (256) | xr (256)]
W12 = M * SS     # layer-1/2 coeff width (m, t, k) with dup cols
O_L2, O_L3 = W12, 2 * W12
CW = 2 * W12 + M * K


def _emit(tc, nc, c1_ap, c2_ap, ps_ap, g1_ap, g2_ap, g3_ap, out_ap):
    vec, act, pool = nc.vector, nc.scalar, nc.gpsimd

    def mk(t, off, dims):
        a = t if isinstance(t, AP) else t[:]
        return AP(a.tensor, a.offset + off, [list(a.ap[0])] + dims)

    def tt_(eng, out, in0, in1, op):
        eng.tensor_tensor(out=out, in0=in0, in1=in1, op=op)

    def clamp(out_ap_, in_ap_):
        vec.tensor_scalar(
            out=out_ap_, in0=in_ap_, scalar1=0.0, scalar2=1.0, op0=AL.max, op1=AL.min
        )

    with (
        tc.tile_pool(name="coef", bufs=1) as pc,
        tc.tile_pool(name="st", bufs=2) as pst,
        tc.tile_pool(name="sr", bufs=2) as psr,
        tc.tile_pool(name="wk", bufs=1) as pwk,
        tc.tile_pool(name="ps", bufs=4, space="PSUM") as pps,
    ):
        # ---- input DMAs, split across the two HW-DGE queues (FIFO per
        # queue: front-load what layer 0 needs).
        tw = pc.tile([P, CW], DT, tag="tw")
        ch1 = pwk.tile([P, CH], DT, tag="ch1")
        ch2 = pwk.tile([P, CH], DT, tag="ch2")
        nc.sync.dma_start(out=ch1[:], in_=c1_ap)
        act.dma_start(out=ch2[:], in_=c2_ap)
        psh = pwk.tile([P, P], DT, tag="psh")
        act.dma_start(out=psh[:], in_=ps_ap)
        nc.sync.dma_start(out=tw[:, 0:W12], in_=g1_ap)
        act.dma_start(out=tw[:, O_L2:O_L2 + W12], in_=g2_ap)
        nc.sync.dma_start(out=tw[:, O_L3:O_L3 + M * K], in_=g3_ap)

        # warm the ACT table bank early so the first PSUM copy-back isn't
        # stuck behind a table load
        scr = pwk.tile([P, 2], F32, tag="scr")
        vec.memset(scr[:], 0.0)
        act.copy(out=scr[:, 1:2], in_=scr[:, 0:1])

        # work tiles (dup-col layout); memset the two whose dup columns
        # are read before ever being written (junk value, but initialized)
        u = pwk.tile([P, 8 * B * SS], DT, tag="u")    # (b, i8, t, k)
        v_t = pwk.tile([P, 4 * B * SS], DT, tag="v")  # (b, j4, t, k)
        w2 = pwk.tile([P, 2 * B * SS], DT, tag="w2")  # (b, j2, t, k)
        tt2 = pwk.tile([P, B * SS], DT, tag="tt")     # (b, t, k)
        vec.memset(v_t[:], 0.0)
        vec.memset(tt2[:], 0.0)

        # ---- layer 0 eval: two 2-D interps into parity planes ---------
        # chunk layout: [c0 cP cQ cPQ | x | xr] (4x64 | 4x64 | 4x64)
        # even half (ch1): s = (c0 + cA*X) + Xr*(cC + cAC*X) -> E plane
        # odd  half (ch2): s = (c0 + cB*Xc) + Xrc*(cD + cBD*Xc) -> O plane
        st1 = pst.tile([P, B * SS], DT, tag="state")
        ue = pwk.tile([P, 2 * B * K], DT, tag="ue")   # (b, s, k)
        te = pwk.tile([P, B * K], DT, tag="te")       # (b, k)

        for half, ch in ((0, ch1), (1, ch2)):
            tt_(vec, mk(ue, 0, [[128, B], [64, 2], [1, K]]),
                mk(ch, 128, [[0, B], [64, 2], [1, K]]),
                mk(ch, 256, [[64, B], [0, 2], [1, K]]), AL.mult)
            tt_(vec, mk(ue, 0, [[128, B], [64, 2], [1, K]]),
                mk(ue, 0, [[128, B], [64, 2], [1, K]]),
                mk(ch, 0, [[0, B], [64, 2], [1, K]]), AL.add)
            tt_(vec, mk(te, 0, [[64, B], [1, K]]),
                mk(ue, 64, [[128, B], [1, K]]),
                mk(ch, 512, [[64, B], [1, K]]), AL.mult)
            tt_(vec, mk(te, 0, [[64, B], [1, K]]),
                mk(te, 0, [[64, B], [1, K]]),
                mk(ue, 0, [[128, B], [1, K]]), AL.add)
            clamp(mk(st1, half * TK, [[SS, B], [1, K]]),
                  mk(te, 0, [[64, B], [1, K]]))
        # fill dup cols: st1(b, t, 64) = st1(b, t, 0)
        vec.tensor_copy(out=mk(st1, K, [[SS, B], [TK, 2], [1, 1]]),
                        in_=mk(st1, 0, [[SS, B], [TK, 2], [1, 1]]))

        # ---- generic layer eval (A, then B, C, D) ---------------------
        NBV = 3   # samples on DVE; sample 3's A level goes to Pool

        def a_level(eng, cofs, st, b0, nb):
            eng.tensor_tensor(
                out=mk(u, b0 * 8 * SS, [[8 * SS, nb], [SS, 8], [1, SS]]),
                in0=mk(tw, cofs + 8 * SS, [[0, nb], [SS, 8], [1, SS]]),
                in1=mk(st, b0 * SS, [[SS, nb], [0, 8], [1, SS]]), op=AL.mult)
            eng.tensor_tensor(
                out=mk(u, b0 * 8 * SS, [[8 * SS, nb], [SS, 8], [1, SS]]),
                in0=mk(u, b0 * 8 * SS, [[8 * SS, nb], [SS, 8], [1, SS]]),
                in1=mk(tw, cofs, [[0, nb], [SS, 8], [1, SS]]), op=AL.add)

        def b_level(st, b0, nb):
            # v_j = u_j + u_{4+j} * Btap; t=0: Btap=O[0..64], t=1: E[1..64]
            tt_(vec, mk(v_t, b0 * 4 * SS, [[4 * SS, nb], [SS, 4], [1, TK]]),
                mk(u, b0 * 8 * SS + 4 * SS, [[8 * SS, nb], [SS, 4], [1, TK]]),
                mk(st, b0 * SS + TK, [[SS, nb], [0, 4], [1, TK]]), AL.mult)
            tt_(vec, mk(v_t, b0 * 4 * SS + TK, [[4 * SS, nb], [SS, 4], [1, K]]),
                mk(u, b0 * 8 * SS + 4 * SS + TK, [[8 * SS, nb], [SS, 4], [1, K]]),
                mk(st, b0 * SS + 1, [[SS, nb], [0, 4], [1, K]]), AL.mult)
            tt_(vec, mk(v_t, b0 * 4 * SS, [[4 * SS, nb], [SS, 4], [1, SS]]),
                mk(v_t, b0 * 4 * SS, [[4 * SS, nb], [SS, 4], [1, SS]]),
                mk(u, b0 * 8 * SS, [[8 * SS, nb], [SS, 4], [1, SS]]), AL.add)

        def eval_layer12(cofs, st, sr, stn):
            a_level(vec, cofs, st, 0, NBV)
            a_level(pool, cofs, st, NBV, B - NBV)
            b_level(st, 0, NBV)
            b_level(st, NBV, B - NBV)
            # C level: w_j = v_j + v_{2+j} * C          (C = sr planes)
            tt_(vec, mk(w2, 0, [[2 * SS, B], [SS, 2], [1, SS]]),
                mk(v_t, 2 * SS, [[4 * SS, B], [SS, 2], [1, SS]]),
                mk(sr, 0, [[SS, B], [0, 2], [1, SS]]), AL.mult)
            tt_(vec, mk(w2, 0, [[2 * SS, B], [SS, 2], [1, SS]]),
                mk(w2, 0, [[2 * SS, B], [SS, 2], [1, SS]]),
                mk(v_t, 0, [[4 * SS, B], [SS, 2], [1, SS]]), AL.add)
            # D level: s = w_0 + w_1 * Dtap  (Dtap like Btap on sr)
            tt_(vec, mk(tt2, 0, [[SS, B], [1, TK]]),
                mk(w2, SS, [[2 * SS, B], [1, TK]]),
                mk(sr, TK, [[SS, B], [1, TK]]), AL.mult)
            tt_(vec, mk(tt2, TK, [[SS, B], [1, K]]),
                mk(w2, SS + TK, [[2 * SS, B], [1, K]]),
                mk(sr, 1, [[SS, B], [1, K]]), AL.mult)
            tt_(vec, mk(tt2, 0, [[SS, B], [1, SS]]),
                mk(tt2, 0, [[SS, B], [1, SS]]),
                mk(w2, 0, [[2 * SS, B], [1, SS]]), AL.add)
            clamp(stn[:], tt2[:])
            # repair dup cols (col 64 is already a true dup; col 129 isn't)
            clamp(mk(stn, TK + K, [[SS, B], [1, 1]]),
                  mk(tt2, TK, [[SS, B], [1, 1]]))

        def rowshift(src):
            # PE permutation matmul (2 x 260) + ACT copy-back from PSUM
            out = psr.tile([P, B * SS], DT, tag="sr")
            for h in (0, 1):
                pt = pps.tile([P, 2 * SS], F32, tag="psum")
                nc.tensor.matmul(pt[:], psh[:],
                                 src[:, h * 2 * SS:(h + 1) * 2 * SS],
                                 start=True, stop=True)
                act.copy(out=out[:, h * 2 * SS:(h + 1) * 2 * SS], in_=pt[:])
            return out

        st = st1
        for l in (1, 2):
            cofs = 0 if l == 1 else O_L2
            sr = rowshift(st)
            stn = pst.tile([P, B * SS], DT, tag="state")
            eval_layer12(cofs, st, sr, stn)
            st = stn

        # ---- layer 3 (even outputs only, plane taps, no dup cols) -----
        sr3 = rowshift(st)
        for b0, nb, eng in ((0, NBV, vec), (NBV, B - NBV, pool)):
            eng.tensor_tensor(
                out=mk(u, b0 * 512, [[512, nb], [64, 8], [1, K]]),
                in0=mk(tw, O_L3 + 8 * K, [[0, nb], [64, 8], [1, K]]),
                in1=mk(st, b0 * SS, [[SS, nb], [0, 8], [1, K]]), op=AL.mult)
            eng.tensor_tensor(
                out=mk(u, b0 * 512, [[512, nb], [64, 8], [1, K]]),
                in0=mk(u, b0 * 512, [[512, nb], [64, 8], [1, K]]),
                in1=mk(tw, O_L3, [[0, nb], [64, 8], [1, K]]), op=AL.add)
        for b0, nb in ((0, NBV), (NBV, B - NBV)):
            tt_(vec, mk(v_t, b0 * 256, [[256, nb], [64, 4], [1, K]]),
                mk(u, b0 * 512 + 256, [[512, nb], [64, 4], [1, K]]),
                mk(st, b0 * SS + TK, [[SS, nb], [0, 4], [1, K]]), AL.mult)
            tt_(vec, mk(v_t, b0 * 256, [[256, nb], [64, 4], [1, K]]),
                mk(v_t, b0 * 256, [[256, nb], [64, 4], [1, K]]),
                mk(u, b0 * 512, [[512, nb], [64, 4], [1, K]]), AL.add)
        tt_(vec, mk(w2, 0, [[128, B], [64, 2], [1, K]]),
            mk(v_t, 128, [[256, B], [64, 2], [1, K]]),
            mk(sr3, 0, [[SS, B], [0, 2], [1, K]]), AL.mult)
        tt_(vec, mk(w2, 0, [[128, B], [64, 2], [1, K]]),
            mk(w2, 0, [[128, B], [64, 2], [1, K]]),
            mk(v_t, 0, [[256, B], [64, 2], [1, K]]), AL.add)
        # D level + output, split by b-halves then quarter DMAs across
        # both HW-DGE queues so the store drains while DVE finishes
        out_t = pwk.tile([P, B * K], DT, tag="out")
        for h in (0, 1):
            o = h * 128          # tt2/out_t half offset (b-stride 64)
            q = h * 256          # w2 half offset (b-stride 128)
            s = h * 2 * SS + TK  # sr3 O-plane half offset
            tt_(vec, mk(tt2, o, [[64, 2], [1, K]]),
                mk(w2, 64 + q, [[128, 2], [1, K]]),
                mk(sr3, s, [[SS, 2], [1, K]]), AL.mult)
            tt_(vec, mk(tt2, o, [[64, 2], [1, K]]),
                mk(tt2, o, [[64, 2], [1, K]]),
                mk(w2, q, [[128, 2], [1, K]]), AL.add)
            clamp(mk(out_t, o, [[64, 1], [1, K]]), mk(tt2, o, [[64, 1], [1, K]]))
            eng = nc.sync if h == 0 else act
            eng.dma_start(out=out_ap[:, o:o + K], in_=out_t[:, o:o + K])
            clamp(mk(out_t, o + K, [[64, 1], [1, K]]),
                  mk(tt2, o + K, [[64, 1], [1, K]]))
            eng.dma_start(out=out_ap[:, o + K:o + 128], in_=out_t[:, o + K:o + 128])


_NC_CACHE = {}


def build():
    if "nc" in _NC_CACHE:
        return _NC_CACHE["nc"]
    nc = bacc.Bacc(
        "TRN2",
        target_bir_lowering=False,
        debug=False,
        enable_asserts=False,
        num_devices=N_CORES,
    )
    c1_d = nc.dram_tensor("ch1", (P, CH), DT, kind="ExternalInput")
    c2_d = nc.dram_tensor("ch2", (P, CH), DT, kind="ExternalInput")
    ps_d = nc.dram_tensor("pshift", (P, P), DT, kind="ExternalInput")
    g1_d = nc.dram_tensor("g1", (P, W12), DT, kind="ExternalInput")
    g2_d = nc.dram_tensor("g2", (P, W12), DT, kind="ExternalInput")
    g3_d = nc.dram_tensor("g3", (P, M * K), DT, kind="ExternalInput")
    out_d = nc.dram_tensor("out", (P, B * K), DT, kind="ExternalOutput")
    with TileContext(nc) as tc:
        _emit(tc, nc, c1_d.ap(), c2_d.ap(), ps_d.ap(), g1_d.ap(), g2_d.ap(),
              g3_d.ap(), out_d.ap())
    nc.compile()
    _NC_CACHE["nc"] = nc
    return nc


def _moebius_coeffs(toggle_gates):
    """sigmoid + Moebius transform of the gate maps -> multilinear coeffs.

    Input-independent weight preprocessing (exact math); returns
    (L, 16, d1, d2) float32 with m = bA*8 + bB*4 + bC*2 + bD.
    """
    tg = np.asarray(toggle_gates, dtype=np.float64)
    c = 1.0 / (1.0 + np.exp(-tg))                       # sigmoid
    c = c.reshape(L, 2, 2, 2, 2, P, Y)                  # (l, bA, bB, bC, bD, x, y)
    for ax in (1, 2, 3, 4):
        hi = [slice(None)] * 7
        lo = [slice(None)] * 7
        hi[ax] = 1
        lo[ax] = 0
        c[tuple(hi)] -= c[tuple(lo)]
    return c.reshape(L, M, P, Y).astype(np.float32)


def make_in_maps(x, toggle_gates):
    x = np.asarray(x, dtype=np.float32)
    c = _moebius_coeffs(toggle_gates)
    # layer 0: only S within {A,C} (even outputs) / {B,D} (odd) survive
    g0e = c[0, [0, 2, 8, 10]][:, :, 0::2]      # [c0, cC, cA, cAC] even cols
    g0o = c[0, [0, 1, 4, 5]][:, :, 1::2]       # [c0, cD, cB, cBD] odd cols
    g0e = g0e.transpose(1, 0, 2).reshape(P, 4 * K)
    g0o = g0o.transpose(1, 0, 2).reshape(P, 4 * K)

    def gl(l):
        a = c[l].transpose(1, 0, 2).reshape(P, M, K, 2)    # (P, m, k, t)
        a = a.transpose(0, 1, 3, 2)                        # (P, m, t, k)
        a = np.concatenate([a, a[..., :1]], axis=-1)       # dup col (TK=65)
        return np.ascontiguousarray(a.reshape(P, W12), dtype=np.float16)

    g1, g2 = gl(1), gl(2)
    g3 = np.ascontiguousarray(
        c[3][:, :, 0::2].transpose(1, 0, 2).reshape(P, M * K), dtype=np.float16)
    psm = np.eye(P, k=-1, dtype=np.float64)
    psm[0, P - 1] = 1.0
    psm = psm.astype(np.float16)
    xr = np.roll(x, -1, axis=1)                            # row shift (x+1)
    ins = []
    for cc in range(N_CORES):
        xs = x[cc * B:(cc + 1) * B]                        # (B, P, K)
        xf = xs.transpose(1, 0, 2).reshape(P, B * K)
        xrf = xr[cc * B:(cc + 1) * B].transpose(1, 0, 2).reshape(P, B * K)
        xcf = np.roll(xf.reshape(P, B, K), -1, axis=2).reshape(P, B * K)
        xrcf = np.roll(xrf.reshape(P, B, K), -1, axis=2).reshape(P, B * K)
        ch1 = np.ascontiguousarray(
            np.concatenate([g0e, xf, xrf], axis=1), dtype=np.float16)
        ch2 = np.ascontiguousarray(
            np.concatenate([g0o, xcf, xrcf], axis=1), dtype=np.float16)
        ins.append({"ch1": ch1, "ch2": ch2, "pshift": psm,
                    "g1": g1, "g2": g2, "g3": g3})
    return ins


def kernel(x, toggle_gates):
    nc = build()
    res = run_bass_kernel_spmd(
        nc, make_in_maps(x, toggle_gates), core_ids=list(range(N_CORES))
    )
    outs = []
    for cc in range(N_CORES):
        o = res.results[cc]["out"].reshape(P, B, K).transpose(1, 0, 2)
        outs.append(o)
    return np.ascontiguousarray(np.concatenate(outs, axis=0), dtype=np.float32)


# revision 6
# speedup vs baseline: 1.1634x; 1.1634x over previous
"""Trainium2 Bass kernel for the soft-logic-gate CA problem.

Math (per sample, grid 128x128, 4 layers):
  state' = clip( sum_m sigmoid(tg[l,m]) * prod_j g(bit_j(m), tap_j), 0, 1 )
  taps: A=state[x,y], B=state[x,y+1], C=state[x+1,y], D=state[x+1,y+1] (periodic)
  g(0,t)=1-t, g(1,t)=t;  m = bA*8 + bB*4 + bC*2 + bD.

4-D multilinear interpolation of the 16 gate maps at corner (A,B,C,D).
The sigmoided gates are converted OFFLINE (host numpy, input-independent
weight preprocessing) to multilinear-polynomial coefficients via the
Moebius transform (c[m] -= c[m-bit]); the device evaluates each layer
with a Horner butterfly of fp16 tensor_tensor ops (A, then B, C, D).

Layout: partition = grid row (128).  State is parity planes (b, t, k):
t=0 even grid cols, t=1 odd.  The periodic column shift y+1 (B/D taps)
is materialized once per layer by the ACT engine into contiguous tap
tiles (stB from state, srB from the rowshift PSUM result), so every DVE
Horner op is a single large 2x-mode tensor_tensor with no wrap-column
splits.  Row shifts (x+1): layer 0 reads host-prepped rolled copies of
x; layers 1-3 use a PE permutation matmul + ACT copy-back.  All
coefficients arrive fp16 in final (m, t, k) layouts: no on-chip
sigmoid, Moebius, or casts.  Layer 0 ships only its 8 live coefficients
(B=D=0 initially), layer 3 only even-column ones.

Sharding: batch 32 -> 8 cores x 4 samples (coefficients replicated).
Engines: DVE does all Horner + clamps (GpSimd stays idle: its SBUF port
is shared with the DVE 2x read port, so concurrent Pool work stalls
DVE); ACT builds tap tiles + PSUM copy-backs; PE the row-shift matmuls;
DMA on the two HW-DGE queues (sync, scalar).
"""

import numpy as np

import concourse.bacc as bacc
import concourse.mybir as mybir
from concourse.ap import AP
from concourse.tile import TileContext
from concourse.bass_utils import run_bass_kernel_spmd

F32 = mybir.dt.float32
DT = mybir.dt.float16  # compute dtype
AL = mybir.AluOpType
P = 128          # partitions = grid rows
B = 4            # samples per core
Y = 128          # grid cols
K = 64           # x cols (even grid cols)
L = 4
M = 16
N_CORES = 8

SK = 2 * K       # state elems per sample (E|O planes)
CH = 6 * 256     # chunk: [g0e g0o | X Xr | Xc Xrc]  (each 4x64)
O_L2, O_L3 = M * Y, 2 * M * Y
CW = 2 * M * Y + M * K


def _emit(tc, nc, ch_ap, ps_ap, g1_ap, g2_ap, g3_ap, out_ap):
    vec, act = nc.vector, nc.scalar

    def mk(t, off, dims):
        a = t if isinstance(t, AP) else t[:]
        return AP(a.tensor, a.offset + off, [list(a.ap[0])] + dims)

    def tt_(eng, out, in0, in1, op):
        eng.tensor_tensor(out=out, in0=in0, in1=in1, op=op)

    def clamp(out_ap_, in_ap_):
        vec.tensor_scalar(
            out=out_ap_, in0=in_ap_, scalar1=0.0, scalar2=1.0, op0=AL.max, op1=AL.min
        )

    with (
        tc.tile_pool(name="coef", bufs=1) as pc,
        tc.tile_pool(name="st", bufs=2) as pst,
        tc.tile_pool(name="sb", bufs=2) as psb,
        tc.tile_pool(name="sr", bufs=2) as psr,
        tc.tile_pool(name="wk", bufs=1) as pwk,
        tc.tile_pool(name="ps", bufs=2, space="PSUM") as pps,
    ):
        # ---- input DMAs, split across the two HW-DGE queues (FIFO per
        # queue: front-load what layer 0 needs).
        tw = pc.tile([P, CW], DT, tag="tw")
        ch = pwk.tile([P, CH], DT, tag="ch")
        nc.sync.dma_start(out=ch[:], in_=ch_ap)
        psh = pwk.tile([P, P], DT, tag="psh")
        act.dma_start(out=psh[:], in_=ps_ap)
        act.dma_start(out=tw[:, 0:M * Y], in_=g1_ap)
        nc.sync.dma_start(out=tw[:, O_L2:O_L2 + M * Y], in_=g2_ap)
        act.dma_start(out=tw[:, O_L3:O_L3 + M * K], in_=g3_ap)

        # warm the ACT table bank early so the first copy isn't stuck
        # behind a table load
        scr = pwk.tile([P, 2], F32, tag="scr")
        vec.memset(scr[:], 0.0)
        act.copy(out=scr[:, 1:2], in_=scr[:, 0:1])

        # ---- layer 0 eval: two 2-D interps into parity planes ---------
        # state layout (b, t, k): b*128 + t*64 + k
        # ch layout: [g0e (4x64) | g0o | X (4x64) | Xr | Xc | Xrc]
        # even half: s = (c0 + cA*X) + Xr*(cC + cAC*X) -> E plane
        # odd  half: s = (c0 + cB*Xc) + Xrc*(cD + cBD*Xc) -> O plane
        st1 = pst.tile([P, B * SK], DT, tag="state")
        ue = pwk.tile([P, 2 * B * K], DT, tag="ue")   # (b, s, k)
        te = pwk.tile([P, B * K], DT, tag="te")       # (b, k)

        for h in (0, 1):
            co, xo = h * 256, 512 + h * 512
            tt_(vec, mk(ue, 0, [[128, B], [64, 2], [1, K]]),
                mk(ch, co + 128, [[0, B], [64, 2], [1, K]]),
                mk(ch, xo, [[64, B], [0, 2], [1, K]]), AL.mult)
            tt_(vec, mk(ue, 0, [[128, B], [64, 2], [1, K]]),
                mk(ue, 0, [[128, B], [64, 2], [1, K]]),
                mk(ch, co, [[0, B], [64, 2], [1, K]]), AL.add)
            tt_(vec, mk(te, 0, [[64, B], [1, K]]),
                mk(ue, 64, [[128, B], [1, K]]),
                mk(ch, xo + 256, [[64, B], [1, K]]), AL.mult)
            tt_(vec, mk(te, 0, [[64, B], [1, K]]),
                mk(te, 0, [[64, B], [1, K]]),
                mk(ue, 0, [[128, B], [1, K]]), AL.add)
            clamp(mk(st1, h * K, [[128, B], [1, K]]), mk(te, 0, [[64, B], [1, K]]))

        # ---- column-shift tap builder (ACT): dst(b,t,k) = y+1 taps ----
        def colshift(src, src_psum=False):
            dst = psb.tile([P, B * SK], DT, tag="stB")
            act.copy(out=mk(dst, 0, [[128, B], [1, K]]),
                     in_=mk(src, 64, [[128, B], [1, K]]))
            act.copy(out=mk(dst, 64, [[128, B], [1, K - 1]]),
                     in_=mk(src, 1, [[128, B], [1, K - 1]]))
            act.copy(out=mk(dst, 127, [[128, B], [1, 1]]),
                     in_=mk(src, 0, [[128, B], [1, 1]]))
            return dst

        # ---- generic layer eval (A, then B, C, D) ---------------------
        u = pwk.tile([P, 8 * B * SK], DT, tag="u")    # (b, i8, t, k)
        v_t = pwk.tile([P, 4 * B * SK], DT, tag="v")  # (b, j4, t, k)
        w2 = pwk.tile([P, 2 * B * SK], DT, tag="w2")  # (b, j2, t, k)
        tt2 = pwk.tile([P, B * SK], DT, tag="tt")     # (b, t, k)

        def eval_layer12(cofs, st, stB, sr, srB, stn):
            # A level: u_i = cLO_i + cHI_i * A
            tt_(vec, mk(u, 0, [[1024, B], [128, 8], [1, 128]]),
                mk(tw, cofs + 8 * Y, [[0, B], [128, 8], [1, 128]]),
                mk(st, 0, [[128, B], [0, 8], [1, 128]]), AL.mult)
            tt_(vec, mk(u, 0, [[1024, B], [128, 8], [1, 128]]),
                mk(u, 0, [[1024, B], [128, 8], [1, 128]]),
                mk(tw, cofs, [[0, B], [128, 8], [1, 128]]), AL.add)
            # B level: v_j = u_j + u_{4+j} * Btap
            tt_(vec, mk(v_t, 0, [[512, B], [128, 4], [1, 128]]),
                mk(u, 512, [[1024, B], [128, 4], [1, 128]]),
                mk(stB, 0, [[128, B], [0, 4], [1, 128]]), AL.mult)
            tt_(vec, mk(v_t, 0, [[512, B], [128, 4], [1, 128]]),
                mk(v_t, 0, [[512, B], [128, 4], [1, 128]]),
                mk(u, 0, [[1024, B], [128, 4], [1, 128]]), AL.add)
            # C level: w_j = v_j + v_{2+j} * C          (C = sr planes)
            tt_(vec, mk(w2, 0, [[256, B], [128, 2], [1, 128]]),
                mk(v_t, 256, [[512, B], [128, 2], [1, 128]]),
                mk(sr, 0, [[128, B], [0, 2], [1, 128]]), AL.mult)
            tt_(vec, mk(w2, 0, [[256, B], [128, 2], [1, 128]]),
                mk(w2, 0, [[256, B], [128, 2], [1, 128]]),
                mk(v_t, 0, [[512, B], [128, 2], [1, 128]]), AL.add)
            # D level: s = w_0 + w_1 * Dtap
            tt_(vec, mk(tt2, 0, [[128, B], [1, 128]]),
                mk(w2, 128, [[256, B], [1, 128]]),
                mk(srB, 0, [[128, B], [1, 128]]), AL.mult)
            tt_(vec, mk(tt2, 0, [[128, B], [1, 128]]),
                mk(tt2, 0, [[128, B], [1, 128]]),
                mk(w2, 0, [[256, B], [1, 128]]), AL.add)
            clamp(stn[:], tt2[:])

        def rowshift(src):
            # PE permutation matmul + ACT copy-back from PSUM; also build
            # the column-shifted D-tap tile from the PSUM result
            pt = pps.tile([P, B * SK], F32, tag="psum")
            nc.tensor.matmul(pt[:], psh[:], src[:], start=True, stop=True)
            out = psr.tile([P, B * SK], DT, tag="sr")
            act.copy(out=out[:], in_=pt[:])
            return out, colshift(pt)

        st = st1
        for l in (1, 2):
            cofs = 0 if l == 1 else O_L2
            sr, srB = rowshift(st)
            stB = colshift(st)
            stn = pst.tile([P, B * SK], DT, tag="state")
            eval_layer12(cofs, st, stB, sr, srB, stn)
            st = stn

        # ---- layer 3 (even outputs only, plane taps, no wraps) --------
        pt3 = pps.tile([P, B * SK], F32, tag="psum")
        nc.tensor.matmul(pt3[:], psh[:], st[:], start=True, stop=True)
        sr3 = psr.tile([P, B * SK], DT, tag="sr")
        act.copy(out=sr3[:], in_=pt3[:])
        tt_(vec, mk(u, 0, [[512, B], [64, 8], [1, K]]),
            mk(tw, O_L3 + 8 * K, [[0, B], [64, 8], [1, K]]),
            mk(st, 0, [[128, B], [0, 8], [1, K]]), AL.mult)
        tt_(vec, mk(u, 0, [[512, B], [64, 8], [1, K]]),
            mk(u, 0, [[512, B], [64, 8], [1, K]]),
            mk(tw, O_L3, [[0, B], [64, 8], [1, K]]), AL.add)
        tt_(vec, mk(v_t, 0, [[256, B], [64, 4], [1, K]]),
            mk(u, 256, [[512, B], [64, 4], [1, K]]),
            mk(st, 64, [[128, B], [0, 4], [1, K]]), AL.mult)
        tt_(vec, mk(v_t, 0, [[256, B], [64, 4], [1, K]]),
            mk(v_t, 0, [[256, B], [64, 4], [1, K]]),
            mk(u, 0, [[512, B], [64, 4], [1, K]]), AL.add)
        tt_(vec, mk(w2, 0, [[128, B], [64, 2], [1, K]]),
            mk(v_t, 128, [[256, B], [64, 2], [1, K]]),
            mk(sr3, 0, [[128, B], [0, 2], [1, K]]), AL.mult)
        tt_(vec, mk(w2, 0, [[128, B], [64, 2], [1, K]]),
            mk(w2, 0, [[128, B], [64, 2], [1, K]]),
            mk(v_t, 0, [[256, B], [64, 2], [1, K]]), AL.add)
        # D level + output: split by b-halves, alternate stores across
        # both HW-DGE queues so the tail store is never queued behind one
        out_t = pwk.tile([P, B * K], DT, tag="out")
        for h in (0, 1):
            o = h * 128          # tt2/out_t half offset (b-stride 64)
            q = h * 256          # w2/sr3 half offset (b-stride 128)
            tt_(vec, mk(tt2, o, [[64, 2], [1, K]]),
                mk(w2, 64 + q, [[128, 2], [1, K]]),
                mk(sr3, 64 + q, [[128, 2], [1, K]]), AL.mult)
            tt_(vec, mk(tt2, o, [[64, 2], [1, K]]),
                mk(tt2, o, [[64, 2], [1, K]]),
                mk(w2, q, [[128, 2], [1, K]]), AL.add)
            clamp(mk(out_t, o, [[64, 1], [1, K]]), mk(tt2, o, [[64, 1], [1, K]]))
            (nc.sync if h == 0 else act).dma_start(
                out=out_ap[:, o:o + K], in_=out_t[:, o:o + K])
            clamp(mk(out_t, o + K, [[64, 1], [1, K]]),
                  mk(tt2, o + K, [[64, 1], [1, K]]))
            (act if h == 0 else nc.sync).dma_start(
                out=out_ap[:, o + K:o + 128], in_=out_t[:, o + K:o + 128])


_NC_CACHE = {}


def build():
    if "nc" in _NC_CACHE:
        return _NC_CACHE["nc"]
    nc = bacc.Bacc(
        "TRN2",
        target_bir_lowering=False,
        debug=False,
        enable_asserts=False,
        num_devices=N_CORES,
    )
    ch_d = nc.dram_tensor("ch", (P, CH), DT, kind="ExternalInput")
    ps_d = nc.dram_tensor("pshift", (P, P), DT, kind="ExternalInput")
    g1_d = nc.dram_tensor("g1", (P, M * Y), DT, kind="ExternalInput")
    g2_d = nc.dram_tensor("g2", (P, M * Y), DT, kind="ExternalInput")
    g3_d = nc.dram_tensor("g3", (P, M * K), DT, kind="ExternalInput")
    out_d = nc.dram_tensor("out", (P, B * K), DT, kind="ExternalOutput")
    with TileContext(nc) as tc:
        _emit(tc, nc, ch_d.ap(), ps_d.ap(), g1_d.ap(), g2_d.ap(),
              g3_d.ap(), out_d.ap())
    nc.compile()
    _NC_CACHE["nc"] = nc
    return nc


def _moebius_coeffs(toggle_gates):
    """sigmoid + Moebius transform of the gate maps -> multilinear coeffs.

    Input-independent weight preprocessing (exact math); returns
    (L, 16, d1, d2) float32 with m = bA*8 + bB*4 + bC*2 + bD.
    """
    tg = np.asarray(toggle_gates, dtype=np.float64)
    c = 1.0 / (1.0 + np.exp(-tg))                       # sigmoid
    c = c.reshape(L, 2, 2, 2, 2, P, Y)                  # (l, bA, bB, bC, bD, x, y)
    for ax in (1, 2, 3, 4):
        hi = [slice(None)] * 7
        lo = [slice(None)] * 7
        hi[ax] = 1
        lo[ax] = 0
        c[tuple(hi)] -= c[tuple(lo)]
    return c.reshape(L, M, P, Y).astype(np.float32)


def make_in_maps(x, toggle_gates):
    x = np.asarray(x, dtype=np.float32)
    c = _moebius_coeffs(toggle_gates)
    # layer 0: only S within {A,C} (even outputs) / {B,D} (odd) survive
    g0e = c[0, [0, 2, 8, 10]][:, :, 0::2]      # [c0, cC, cA, cAC] even cols
    g0o = c[0, [0, 1, 4, 5]][:, :, 1::2]       # [c0, cD, cB, cBD] odd cols
    g0e = g0e.transpose(1, 0, 2).reshape(P, 4 * K)
    g0o = g0o.transpose(1, 0, 2).reshape(P, 4 * K)

    def gl(l):
        a = c[l].transpose(1, 0, 2).reshape(P, M, K, 2)    # (P, m, k, t)
        return np.ascontiguousarray(
            a.transpose(0, 1, 3, 2).reshape(P, M * Y), dtype=np.float16)

    g1, g2 = gl(1), gl(2)
    g3 = np.ascontiguousarray(
        c[3][:, :, 0::2].transpose(1, 0, 2).reshape(P, M * K), dtype=np.float16)
    psm = np.eye(P, k=-1, dtype=np.float64)
    psm[0, P - 1] = 1.0
    psm = psm.astype(np.float16)
    xr = np.roll(x, -1, axis=1)                            # row shift (x+1)
    ins = []
    for cc in range(N_CORES):
        xs = x[cc * B:(cc + 1) * B]                        # (B, P, K)
        xf = xs.transpose(1, 0, 2).reshape(P, B * K)
        xrf = xr[cc * B:(cc + 1) * B].transpose(1, 0, 2).reshape(P, B * K)
        xcf = np.roll(xf.reshape(P, B, K), -1, axis=2).reshape(P, B * K)
        xrcf = np.roll(xrf.reshape(P, B, K), -1, axis=2).reshape(P, B * K)
        ch = np.ascontiguousarray(
            np.concatenate([g0e, g0o, xf, xrf, xcf, xrcf], axis=1),
            dtype=np.float16)
        ins.append({"ch": ch, "pshift": psm, "g1": g1, "g2": g2, "g3": g3})
    return ins


def kernel(x, toggle_gates):
    nc = build()
    res = run_bass_kernel_spmd(
        nc, make_in_maps(x, toggle_gates), core_ids=list(range(N_CORES))
    )
    outs = []
    for cc in range(N_CORES):
        o = res.results[cc]["out"].reshape(P, B, K).transpose(1, 0, 2)
        outs.append(o)
    return np.ascontiguousarray(np.concatenate(outs, axis=0), dtype=np.float32)
